# revision 15
# baseline (speedup 1.0000x reference)
"""Single-head attention kernel for Trainium2 (Bass/Tile), 8 NeuronCores.

Problem: B=4, S=4096, D=1024, H=128 fp32.
    q,k,v = x @ W{q,k,v};  out = softmax(q k^T / sqrt(H)) @ v

Sharding: 8 cores = (batch b, KEY-half kh).  Each core computes PARTIAL
attention for all 4096 queries over its 2048 keys; the host combines the
two partial results per batch: out = (outT_0 + outT_1) / (l_0 + l_1).
The host permutes each core's x rows so its key rows come first and
transposes/casts to xT fp16; query order follows the permutation.

fp16 on all matmul operands (fp8 measurably breaks the max-rel-err budget
on low-entropy softmax rows), fp32 accumulation in PSUM.

v2.1 design (from trace evidence):
  - x is staged per 512-row slice as one CONTIGUOUS 8KB/partition block:
    a slice DMA is 128 descriptors instead of 1024+, so the Sync queue's
    descriptor generation (~3us/slice in v2) stops gating the input.
  - PE p-state: any PE idle gap drops the clock to ~1.2GHz for ~3us
    (HAM).  The warm-up matmul count is sized to bridge from wq arrival
    to slice-0 arrival so the PE never idles once real work starts.
  - ScalarE does ONLY the 64 exps (the ~65us floor).  Softmax
    denominators: DVE/GpSimd pairwise add-tree per chunk, lopsided so
    only ONE add trails the last exp; chunks 0-2 reduce across
    partitions with GpSimd partition_all_reduce (6.9us, overlapped into
    the next chunk), the last chunk uses a ones-matmul on the (by then
    idle) PE + ScalarE copies so the tail stays short.
  - Projections beyond q0,k0,q1 are emitted just-in-time inside chunk 0
    (k_g before score(4g), v_g before AV(4g)); q2..q7 land at the ends
    of chunks 0-2.  outT is evacuated in 512-halves before the tree tail
    so the next chunk's first AV is not blocked.
"""

import math

import numpy as np

import concourse.bacc as bacc
import concourse.bass_isa as bass_isa
import concourse.mybir as mybir
import concourse.tile as tile
from concourse.bass_utils import run_bass_kernel_spmd

B, S, D, H = 4, 4096, 1024, 128
NCORES = 8
SK = S // 2  # keys per core (2048)
RB = 512  # rows per projection block
NRB = S // RB  # 8 query blocks
NKRB = SK // RB  # 4 key blocks
QC = 1024  # queries per attention chunk
NQC = S // QC  # 4 chunks
NKB = SK // 128  # 16 key blocks of 128
NDC = D // 128  # 8 contraction chunks

F32 = mybir.dt.float32
F16 = mybir.dt.float16
F8 = mybir.dt.float8e4  # TRN e4m3: max normal 240, Inf at 256

_CACHE = {}


def build_nc():
    nc = bacc.Bacc("TRN2", target_bir_lowering=False, debug=False)

    # x slices pre-packed contiguous: [g, p, c*RB+s] = x[c*128+p, g*RB+s]
    xt_d = nc.dram_tensor("xt", [NRB, 128, NDC * RB], F16, kind="ExternalInput")
    # weights host-preswizzled to [128, NDC*H]: row p, chunk c = W[c*128+p, :]
    wq_d = nc.dram_tensor("wq", [128, NDC * H], F16, kind="ExternalInput")
    wk_d = nc.dram_tensor("wk", [128, NDC * H], F16, kind="ExternalInput")
    wv_d = nc.dram_tensor("wv", [128, NDC * H], F16, kind="ExternalInput")
    ident_d = nc.dram_tensor("ident", [128, 128], F16, kind="ExternalInput")
    # partial (key-shard) unnormalized out^T [h, q] and denominators l [1, q]
    # fp16 partial sums: ~5e-4 relative on the unnormalized numerators,
    # negligible after host normalization; half the output DMA traffic
    outT_d = nc.dram_tensor("outT", [H, S], F16, kind="ExternalOutput")
    # every chunk's l leaves as the UNREDUCED [128, QC] tree sum; the host
    # does the partition sums.  This removes the GpSimd partition_all_reduce
    # chain (tail dependency + SBUF port contention) for 1MB extra output DMA
    # that rides the queue's idle windows.
    lfin_d = nc.dram_tensor("lfin", [128, S], F16, kind="ExternalOutput")

    scale = 1.0 / math.sqrt(H)

    with tile.TileContext(nc) as tc:
        with (
            tc.tile_pool(name="const", bufs=1) as constp,
            tc.tile_pool(name="persist", bufs=1) as persist,
            tc.tile_pool(name="attn", bufs=8) as attn_pool,
            tc.tile_pool(name="tree", bufs=2) as tree_pool,
            tc.tile_pool(name="fin", bufs=2) as fin_pool,
            tc.tile_pool(name="ps_p", bufs=2, space="PSUM") as ps_p,
            tc.tile_pool(name="ps_s", bufs=2, space="PSUM") as ps_s,
            tc.tile_pool(name="ps_o", bufs=1, space="PSUM") as ps_o,
        ):
            # ---- DMA, ordered for the critical path ----
            w_sb = {}
            for name in ("wq", "wk", "wv"):
                w_sb[name] = constp.tile([128, NDC, H], F16, name=f"{name}_sb")

            def load_w(name):
                nc.sync.dma_start(
                    w_sb[name][:],
                    {"wq": wq_d, "wk": wk_d, "wv": wv_d}[name]
                    .ap()
                    .rearrange("p (c h) -> p c h", c=NDC),
                )

            xt_sb = persist.tile([128, NDC, S], F16, name="xt_sb")

            def load_slice(g, eng=None):
                (eng or nc.sync).dma_start(
                    xt_sb[:, :, g * RB : (g + 1) * RB],
                    xt_d.ap()[g].rearrange("p (c s) -> p c s", c=NDC),
                )

            ident = constp.tile([128, 128], F16, name="ident_sb")
            ones = constp.tile([128, 1], F16, name="ones_sb")
            nc.vector.memset(ones[:], 1.0)

            # head order: wq then slice0 (the two gates for the first real
            # projection).  slice0 rides the SCALAR engine's hardware DMA
            # queue so it streams concurrently with wq/wk/slice1 on the Sync
            # queue instead of serializing behind them.
            load_w("wq")
            load_slice(0, eng=nc.scalar)
            load_w("wk")
            load_slice(1)
            load_w("wv")
            nc.sync.dma_start(ident[:], ident_d.ap())
            for g in range(2, NRB):
                load_slice(g)

            # ---- persistent activations ----
            qt_sb = persist.tile([128, S], F16, name="qt_sb")  # [h, q] all q
            kt_sb = persist.tile([128, SK], F16, name="kt_sb")  # [h, k] own
            v_sb = persist.tile([128, NKB, H], F16, name="v_sb")  # own keys
            vt_sb = persist.tile([128, SK], F16, name="vt_sb")  # staging

            # preload the exp table during the input DMA
            warm = constp.tile([1, 1], F32, name="warm_sb")
            nc.scalar.activation(
                warm[:], ones[0:1, :], mybir.ActivationFunctionType.Exp
            )
            # HAM warm-up sized to bridge wq-arrival -> slice0-arrival so the
            # PE is at 2.4 GHz and BUSY when the first projection can start
            # (trace: wq lands ~10.4us; slice0 on the parallel Scalar queue
            # ~12.6us; 20 x ~107ns cold matmuls spans that gap)
            NWARM = 20
            warm_ps = ps_p.tile([128, 128], F32, tag="proj")
            for i in range(NWARM):
                nc.tensor.matmul(
                    warm_ps[:],
                    w_sb["wq"][:, 0, :],
                    w_sb["wq"][:, 0, :],
                    start=(i == 0),
                    stop=(i == NWARM - 1),
                )

            def project(wname, dst_sb, rb):
                """One 512-row projection block through one proj PSUM bank."""
                ps = ps_p.tile([128, RB], F32, tag="proj")
                for dc in range(NDC):
                    nc.tensor.matmul(
                        ps[:],
                        w_sb[wname][:, dc, :],
                        xt_sb[:, dc, rb * RB : (rb + 1) * RB],
                        start=(dc == 0),
                        stop=(dc == NDC - 1),
                    )
                nc.vector.tensor_copy(dst_sb[:, rb * RB : (rb + 1) * RB], ps[:])

            def project_spread(wname, dst_sb, rb, piece=2):
                """Same projection, emitted as NDC//piece separate steps so
                the PE work interleaves the score/AV stream finely instead of
                stalling the exp cadence with an 8-matmul lump."""
                state = {"dc": 0, "ps": None}

                def step():
                    if state["ps"] is None:
                        state["ps"] = ps_p.tile(
                            [128, RB], F32, tag="proj", name="qspread_ps"
                        )
                    ps = state["ps"]
                    for _ in range(piece):
                        dc = state["dc"]
                        nc.tensor.matmul(
                            ps[:],
                            w_sb[wname][:, dc, :],
                            xt_sb[:, dc, rb * RB : (rb + 1) * RB],
                            start=(dc == 0),
                            stop=(dc == NDC - 1),
                        )
                        state["dc"] += 1
                    if state["dc"] == NDC:
                        nc.vector.tensor_copy(
                            dst_sb[:, rb * RB : (rb + 1) * RB], ps[:]
                        )

                return step

            def v_transpose(g):
                v_ps = ps_p.tile([128, RB], F16, tag="proj")
                for s in range(4):
                    nc.tensor.transpose(
                        v_ps[:, s * 128 : (s + 1) * 128],
                        vt_sb[:, g * RB + s * 128 : g * RB + (s + 1) * 128],
                        ident[:],
                    )
                nc.vector.tensor_copy(
                    v_sb[:, g * 4 : (g + 1) * 4, :].rearrange("p a b -> p (a b)"),
                    v_ps[:, 0 : 4 * H],
                )

            # Front: the minimum needed for the first score matmul.
            project("wq", qt_sb, 0)
            project("wk", kt_sb, 0)
            project("wq", qt_sb, 1)

            # Just-in-time projection emission points: work emitted at slot j
            # runs before score(j+1)/AV(j), so k_g must sit at slot < 4g,
            # v_g at slot < 4g (first use AV(4g)), q2/q3 before chunk 1.
            def do_work(item):
                kind, g = item
                if kind == "q":
                    project("wq", qt_sb, g)
                elif kind == "k":
                    project("wk", kt_sb, g)
                else:
                    project("wv", vt_sb, g)
                    v_transpose(g)

            # chunk 0: whole blocks just-in-time (k_g < score(4g), v_g <
            # AV(4g)); q2/q3 pulled off the chunk tail so chunk 1's first
            # score is not queued behind them.
            chunk0_work = {
                0: [("v", 0)],
                1: [("k", 1)],
                3: [("v", 1)],
                5: [("k", 2)],
                7: [("v", 2)],
                9: [("k", 3)],
                10: [("q", 2)],
                11: [("v", 3)],
                12: [("q", 3)],
            }

            # ---- attention: per chunk, 16 kb of score->exp->AV + l-tree ----
            for qcidx in range(NQC):
                outT_ps = ps_o.tile([128, QC], F32, tag="outT")
                at_tiles = {}
                pairs = {}
                tree_t = {}

                def score(kb):
                    st_ps = ps_s.tile([128, QC], F32, tag="st")
                    for h in range(QC // 512):
                        nc.tensor.matmul(
                            st_ps[:, h * 512 : (h + 1) * 512],
                            kt_sb[:, kb * 128 : (kb + 1) * 128],
                            qt_sb[
                                :, qcidx * QC + h * 512 : qcidx * QC + (h + 1) * 512
                            ],
                            start=True,
                            stop=True,
                        )
                    at_sb = attn_pool.tile([128, QC], F16, tag="at", name="at_sb")
                    nc.scalar.activation(
                        at_sb[:],
                        st_ps[:],
                        mybir.ActivationFunctionType.Exp,
                        scale=scale,
                    )
                    at_tiles[kb] = at_sb

                def accum_av(kb):
                    at_sb = at_tiles[kb]
                    for h in range(QC // 512):
                        nc.tensor.matmul(
                            outT_ps[:, h * 512 : (h + 1) * 512],
                            v_sb[:, kb, :],
                            at_sb[:, h * 512 : (h + 1) * 512],
                            start=(kb == 0),
                            stop=(kb == NKB - 1),
                        )

                def tadd(name, a, b, eng):
                    t = tree_pool.tile([128, QC], F16, tag=name)
                    eng.tensor_add(t[:], a[:], b[:])
                    return t

                def tree(kb):
                    # Lopsided pairwise reduction of the 16 at tiles,
                    # restructured so only ONE DVE add trails exp(15):
                    # part3 = part2 + at14 forms as soon as exp(14) lands.
                    if kb % 2 == 1 and kb < 15:
                        p = kb // 2
                        pairs[p] = tadd(
                            "pair", at_tiles[kb - 1], at_tiles[kb], nc.vector
                        )
                        at_tiles.pop(kb - 1)
                    if kb == 3:
                        tree_t["quad0"] = tadd("quad0", pairs[0], pairs[1], nc.vector)
                    if kb == 7:
                        tree_t["quad1"] = tadd("quad1", pairs[2], pairs[3], nc.vector)
                        tree_t["oct0"] = tadd(
                            "oct0", tree_t["quad0"], tree_t["quad1"], nc.vector
                        )
                    if kb == 11:
                        tree_t["quad2"] = tadd("quad2", pairs[4], pairs[5], nc.vector)
                        tree_t["part1"] = tadd(
                            "part1", tree_t["oct0"], tree_t["quad2"], nc.vector
                        )
                    if kb == 13:
                        tree_t["part2"] = tadd(
                            "part2", tree_t["part1"], pairs[6], nc.vector
                        )
                    if kb == 14:
                        tree_t["part3"] = tadd(
                            "part3", tree_t["part2"], at_tiles[14], nc.vector
                        )
                    if kb == 15:
                        final = tadd("final", tree_t["part3"], at_tiles[15], nc.vector)
                        # ship the unreduced [128, QC] tree sum; host finishes
                        nc.sync.dma_start(
                            lfin_d.ap()[:, qcidx * QC : (qcidx + 1) * QC],
                            final[:],
                        )

                if qcidx == 0:
                    work = {
                        kb: [lambda item=item: do_work(item) for item in items]
                        for kb, items in chunk0_work.items()
                    }
                elif qcidx < NQC - 1:
                    # next chunk's two q blocks, emitted as 2-dc slivers so
                    # the exp cadence sees no projection lump
                    qa = project_spread("wq", qt_sb, 2 * qcidx + 2)
                    qb = project_spread("wq", qt_sb, 2 * qcidx + 3)
                    work = {1: [qa], 3: [qa], 5: [qa], 7: [qa],
                            9: [qb], 11: [qb], 13: [qb], 15: [qb]}
                else:
                    work = {}
                score(0)
                for kb in range(1, NKB):
                    # chunk 0's k/v JIT blocks must precede their score; the
                    # q-spread slivers of later chunks go AFTER the score so
                    # the exp cadence is never queued behind a projection
                    if qcidx == 0:
                        for thunk in work.get(kb - 1, ()):
                            thunk()
                        score(kb)
                    else:
                        score(kb)
                        for thunk in work.get(kb - 1, ()):
                            thunk()
                    accum_av(kb - 1)
                    tree(kb - 1)
                for thunk in work.get(NKB - 1, ()):
                    thunk()
                accum_av(NKB - 1)

                # evacuate outT in halves (DVE) so the next chunk's first AV
                # is not blocked behind the full copy.  For the LAST chunk the
                # l path is the longer tail chain, so it goes first there.
                def evac_outT():
                    outT_sb = fin_pool.tile([128, QC], F16, tag="outT_sb")
                    for h in range(QC // 512):
                        nc.vector.tensor_copy(
                            outT_sb[:, h * 512 : (h + 1) * 512],
                            outT_ps[:, h * 512 : (h + 1) * 512],
                        )
                        nc.sync.dma_start(
                            outT_d.ap()[
                                :, qcidx * QC + h * 512 : qcidx * QC + (h + 1) * 512
                            ],
                            outT_sb[:, h * 512 : (h + 1) * 512],
                        )

                if qcidx < NQC - 1:
                    evac_outT()
                    tree(NKB - 1)
                else:
                    tree(NKB - 1)
                    evac_outT()

    nc.compile()
    return nc


def _get_nc():
    if "nc" not in _CACHE:
        _CACHE["nc"] = build_nc()
    return _CACHE["nc"]


def _swizzle_w(W):
    # [D, H] -> [128, NDC*H]: row p, chunk c holds W[c*128+p, :]
    W = np.asarray(W, dtype=np.float16)
    return np.ascontiguousarray(
        W.reshape(NDC, 128, H).transpose(1, 0, 2).reshape(128, NDC * H)
    )


def make_in_maps(inputs, Wq, Wk, Wv):
    inputs = np.asarray(inputs, dtype=np.float32)
    Wq = _swizzle_w(Wq)
    Wk = _swizzle_w(Wk)
    Wv = _swizzle_w(Wv)
    ident = np.eye(128, dtype=np.float16)

    in_maps = []
    for c in range(NCORES):
        b, kh = divmod(c, 2)
        xb = inputs[b]
        # own key-half rows first; queries follow the same permutation
        xk = np.concatenate(
            [xb[kh * SK : (kh + 1) * SK], xb[(1 - kh) * SK : (2 - kh) * SK]], axis=0
        )
        xt = xk.T.astype(np.float16)  # [D, S]
        # slice-contiguous staging: [g, p, c*RB+s] = xT[c*128+p, g*RB+s]
        xt8 = np.ascontiguousarray(
            xt.reshape(NDC, 128, NRB, RB)
            .transpose(2, 1, 0, 3)
            .reshape(NRB, 128, NDC * RB)
        )
        in_maps.append(
            {
                "xt": xt8,
                "wq": Wq,
                "wk": Wk,
                "wv": Wv,
                "ident": ident,
            }
        )
    return in_maps


def kernel(inputs, Wq, Wk, Wv):
    nc = _get_nc()
    in_maps = make_in_maps(inputs, Wq, Wk, Wv)

    res = run_bass_kernel_spmd(nc, in_maps, core_ids=list(range(NCORES)))

    out = np.empty((B, S, H), dtype=np.float32)
    for b in range(B):
        num = np.zeros((H, S), dtype=np.float32)
        den = np.zeros((1, S), dtype=np.float32)
        for kh in range(2):
            c = 2 * b + kh
            outT = res.results[c]["outT"].astype(np.float32)  # [H,S], permuted
            # denominators: host-side partition sum of the kernel's
            # unreduced [128, S] tree totals
            l = (
                res.results[c]["lfin"].astype(np.float32).sum(axis=0)[None, :]
            )  # [1, S]
            # queries were ordered [kh-half, other-half]; map back
            perm = np.concatenate(
                [
                    np.arange(kh * SK, (kh + 1) * SK),
                    np.arange((1 - kh) * SK, (2 - kh) * SK),
                ]
            )
            num[:, perm] += outT
            den[:, perm] += l
        out[b] = (num / den).T
    return out



# revision 18
# speedup vs baseline: 1.0601x; 1.0601x over previous
"""Single-head attention kernel for Trainium2 (Bass/Tile), 8 NeuronCores.

Problem: B=4, S=4096, D=1024, H=128 fp32.
    q,k,v = x @ W{q,k,v};  out = softmax(q k^T / sqrt(H)) @ v

Sharding: 8 cores = (batch b, KEY-half kh).  Each core computes PARTIAL
attention for all 4096 queries over its 2048 keys; the host combines the
two partial results per batch: out = (outT_0 + outT_1) / (l_0 + l_1).
The host permutes each core's x rows so its key rows come first and
transposes/casts to xT fp16; query order follows the permutation.

fp16 on all matmul operands, fp32 accumulation in PSUM.  fp8 (e4m3,
DoubleRow) was tried for the AV matmul and measurably fails the budget:
with the max/rms-rel metric, out = sum_k p_k v_k has rms(out) =
sqrt(sum p^2) while the fp8 noise contributes eps*sqrt(sum p^2) -- the
averaging cancels in the ratio, so out inherits fp8's ~2% element error
directly (measured 2.2e-2 vs the 2e-2 budget, no speedup either since
the exp chain becomes the bottleneck).

v3 design (from trace evidence):
  - x is staged per 512-row slice as one CONTIGUOUS 8KB/partition block
    (128 descriptors/slice).  Head order: wq, slice0, wk, slice1, wv,
    ident, slices 2-7 -- wq and slice0 gate the first real projection.
    A parallel per-engine DMA queue for slice0 is NOT faster (secondary
    queues get fewer DMA engines, ~229GB/s).
  - PE p-state: any PE idle gap drops the clock to ~1.2GHz for ~3.4us
    (HAM).  The warm-up matmul count bridges wq arrival (~10.4us) to
    slice-0 arrival (~14.3us) so the PE never idles once real work
    starts and the clock is at 2.4GHz from the first projection.
  - ScalarE does ONLY the 64 exps at (1024+352)/1.2 = ~1147ns each --
    the ~73us floor.  Chunk 0 is PE-bound (projections), chunks 1-3 are
    exp-bound; the mid-section sits near the dependency floor.
  - Flat 64-slot schedule: slot s emits score(s) then the AV for slot
    s-LAG (LAG=4).  The AV lag crosses chunk boundaries so chunk c+1's
    scores run while chunk c's last AVs + outT evac drain (no PE stall
    on the ps_o reuse), and chunk 3's exp-bound idle absorbs the debt.
  - Denominators: DVE pairwise add-tree per chunk, lopsided with
    part3 = part2 + at14 at exp(14) so only ONE add trails exp(15);
    every chunk ships its UNREDUCED [128, QC] fp16 tree sum and the
    host does the cross-partition sums (kills the GpSimd
    partition_all_reduce chain and its tail dependency).
  - Chunk-0 JIT: k_g before score(4g), v_g before AV(4g) (at slot
    4g+LAG), q2/q3 before chunk 1; chunks 1-2 spread the next chunk's
    q blocks as 2-dc slivers emitted AFTER each score.
"""

import math

import numpy as np

import concourse.bacc as bacc
import concourse.bass_isa as bass_isa
import concourse.mybir as mybir
import concourse.tile as tile
from concourse.bass_utils import run_bass_kernel_spmd

B, S, D, H = 4, 4096, 1024, 128
NCORES = 8
SK = S // 2  # keys per core (2048)
RB = 512  # rows per projection block
NRB = S // RB  # 8 query blocks
NKRB = SK // RB  # 4 key blocks
QC = 1024  # queries per attention chunk
NQC = S // QC  # 4 chunks
NKB = SK // 128  # 16 key blocks of 128
NDC = D // 128  # 8 contraction chunks

F32 = mybir.dt.float32
F16 = mybir.dt.float16
F8 = mybir.dt.float8e4  # TRN e4m3: max normal 240, Inf at 256

_CACHE = {}


def build_nc():
    nc = bacc.Bacc("TRN2", target_bir_lowering=False, debug=False)

    # x slices pre-packed contiguous: [g, p, c*RB+s] = x[c*128+p, g*RB+s]
    xt_d = nc.dram_tensor("xt", [NRB, 128, NDC * RB], F16, kind="ExternalInput")
    # weights host-preswizzled to [128, NDC*H]: row p, chunk c = W[c*128+p, :]
    wq_d = nc.dram_tensor("wq", [128, NDC * H], F16, kind="ExternalInput")
    wk_d = nc.dram_tensor("wk", [128, NDC * H], F16, kind="ExternalInput")
    wv_d = nc.dram_tensor("wv", [128, NDC * H], F16, kind="ExternalInput")
    ident_d = nc.dram_tensor("ident", [128, 128], F16, kind="ExternalInput")
    # partial (key-shard) unnormalized out^T [h, q] and denominators l [1, q]
    # fp16 partial sums: ~5e-4 relative on the unnormalized numerators,
    # negligible after host normalization; half the output DMA traffic
    outT_d = nc.dram_tensor("outT", [H, S], F16, kind="ExternalOutput")
    # every chunk's l leaves as the UNREDUCED [128, QC] tree sum; the host
    # does the partition sums.  This removes the GpSimd partition_all_reduce
    # chain (tail dependency + SBUF port contention) for 1MB extra output DMA
    # that rides the queue's idle windows.
    lfin_d = nc.dram_tensor("lfin", [128, S], F16, kind="ExternalOutput")

    scale = 1.0 / math.sqrt(H)

    with tile.TileContext(nc) as tc:
        with (
            tc.tile_pool(name="const", bufs=1) as constp,
            tc.tile_pool(name="persist", bufs=1) as persist,
            tc.tile_pool(name="attn", bufs=8) as attn_pool,
            tc.tile_pool(name="tree", bufs=2) as tree_pool,
            tc.tile_pool(name="fin", bufs=2) as fin_pool,
            tc.tile_pool(name="ps_p", bufs=2, space="PSUM") as ps_p,
            tc.tile_pool(name="ps_s", bufs=2, space="PSUM") as ps_s,
            tc.tile_pool(name="ps_o", bufs=1, space="PSUM") as ps_o,
        ):
            # ---- DMA, ordered for the critical path ----
            w_sb = {}
            for name in ("wq", "wk", "wv"):
                w_sb[name] = constp.tile([128, NDC, H], F16, name=f"{name}_sb")

            def load_w(name):
                nc.sync.dma_start(
                    w_sb[name][:],
                    {"wq": wq_d, "wk": wk_d, "wv": wv_d}[name]
                    .ap()
                    .rearrange("p (c h) -> p c h", c=NDC),
                )

            xt_sb = persist.tile([128, NDC, S], F16, name="xt_sb")

            def load_slice(g, eng=None):
                (eng or nc.sync).dma_start(
                    xt_sb[:, :, g * RB : (g + 1) * RB],
                    xt_d.ap()[g].rearrange("p (c s) -> p c s", c=NDC),
                )

            ident = constp.tile([128, 128], F16, name="ident_sb")
            ones = constp.tile([128, 1], F16, name="ones_sb")
            nc.vector.memset(ones[:], 1.0)

            # head order: wq then slice0 (the two gates for the first real
            # projection), then wk/slice1/wv/ident, then the remaining slices.
            # (A parallel per-engine DMA queue for slice0 was tried and is NOT
            # faster: secondary queues get fewer DMA engines, ~229GB/s, so
            # slice0 lands at the same ~14.3us while the PE idle-gap resets
            # the HAM activity window and costs ~2us of cold clock.)
            load_w("wq")
            load_slice(0)
            load_w("wk")
            load_slice(1)
            load_w("wv")
            nc.sync.dma_start(ident[:], ident_d.ap())
            for g in range(2, NRB):
                load_slice(g)

            # ---- persistent activations ----
            qt_sb = persist.tile([128, S], F16, name="qt_sb")  # [h, q] all q
            kt_sb = persist.tile([128, SK], F16, name="kt_sb")  # [h, k] own
            v_sb = persist.tile([128, NKB, H], F16, name="v_sb")  # own keys
            vt_sb = persist.tile([128, SK], F16, name="vt_sb")  # staging

            # preload the exp table during the input DMA
            warm = constp.tile([1, 1], F32, name="warm_sb")
            nc.scalar.activation(
                warm[:], ones[0:1, :], mybir.ActivationFunctionType.Exp
            )
            # HAM warm-up sized to bridge wq-arrival -> slice0-arrival so the
            # PE is at 2.4 GHz and BUSY when the first projection can start
            # (trace: wq lands ~10.4us, slice0 ~14.3us; 32 x ~107ns cold
            # matmuls spans that gap without over-spinning past it)
            NWARM = 32
            warm_ps = ps_p.tile([128, 128], F32, tag="proj")
            for i in range(NWARM):
                nc.tensor.matmul(
                    warm_ps[:],
                    w_sb["wq"][:, 0, :],
                    w_sb["wq"][:, 0, :],
                    start=(i == 0),
                    stop=(i == NWARM - 1),
                )

            def project(wname, dst_sb, rb):
                """One 512-row projection block through one proj PSUM bank."""
                ps = ps_p.tile([128, RB], F32, tag="proj")
                for dc in range(NDC):
                    nc.tensor.matmul(
                        ps[:],
                        w_sb[wname][:, dc, :],
                        xt_sb[:, dc, rb * RB : (rb + 1) * RB],
                        start=(dc == 0),
                        stop=(dc == NDC - 1),
                    )
                nc.vector.tensor_copy(dst_sb[:, rb * RB : (rb + 1) * RB], ps[:])

            def project_spread(wname, dst_sb, rb, piece=2):
                """Same projection, emitted as NDC//piece separate steps so
                the PE work interleaves the score/AV stream finely instead of
                stalling the exp cadence with an 8-matmul lump."""
                state = {"dc": 0, "ps": None}

                def step():
                    if state["ps"] is None:
                        state["ps"] = ps_p.tile(
                            [128, RB], F32, tag="proj", name="qspread_ps"
                        )
                    ps = state["ps"]
                    for _ in range(piece):
                        dc = state["dc"]
                        nc.tensor.matmul(
                            ps[:],
                            w_sb[wname][:, dc, :],
                            xt_sb[:, dc, rb * RB : (rb + 1) * RB],
                            start=(dc == 0),
                            stop=(dc == NDC - 1),
                        )
                        state["dc"] += 1
                    if state["dc"] == NDC:
                        nc.vector.tensor_copy(
                            dst_sb[:, rb * RB : (rb + 1) * RB], ps[:]
                        )

                return step

            def v_transpose(g):
                v_ps = ps_p.tile([128, RB], F16, tag="proj")
                for s in range(4):
                    nc.tensor.transpose(
                        v_ps[:, s * 128 : (s + 1) * 128],
                        vt_sb[:, g * RB + s * 128 : g * RB + (s + 1) * 128],
                        ident[:],
                    )
                nc.vector.tensor_copy(
                    v_sb[:, g * 4 : (g + 1) * 4, :].rearrange("p a b -> p (a b)"),
                    v_ps[:, 0 : 4 * H],
                )

            # Front: the minimum needed for the first score matmul.
            project("wq", qt_sb, 0)
            project("wk", kt_sb, 0)
            project("wq", qt_sb, 1)

            # Just-in-time projection emission points: work emitted at slot j
            # runs before score(j+1)/AV(j), so k_g must sit at slot < 4g,
            # v_g at slot < 4g (first use AV(4g)), q2/q3 before chunk 1.
            def do_work(item):
                kind, g = item
                if kind == "q":
                    project("wq", qt_sb, g)
                elif kind == "k":
                    project("wk", kt_sb, g)
                else:
                    project("wv", vt_sb, g)
                    v_transpose(g)

            # chunk 0: whole blocks just-in-time (k_g < score(4g), v_g <
            # AV(4g)); q2/q3 pulled off the chunk tail so chunk 1's first
            # score is not queued behind them.
            chunk0_work = {
                0: [("v", 0)],
                1: [("k", 1)],
                3: [("v", 1)],
                5: [("k", 2)],
                7: [("v", 2)],
                9: [("k", 3)],
                10: [("q", 2)],
                11: [("v", 3)],
                12: [("q", 3)],
            }

            # ---- attention: flat 64-slot schedule with cross-chunk AV lag ----
            # Slot s = (chunk c = s//16, kb = s%16): emit score(s), then the
            # AV for slot s-LAG.  The lag CROSSES chunk boundaries, so chunk
            # c+1's scores start while chunk c's last AVs + outT evac drain:
            # no PE stall on the ps_o reuse at chunk starts, and chunk 3's
            # exp-bound PE idle absorbs the lag debt at the end.
            LAG = 4
            NS = NQC * NKB
            states = {}

            def get_state(c):
                if c not in states:
                    states[c] = {
                        "outT": ps_o.tile(
                            [128, QC], F32, tag="outT", name="outT_ps"
                        ),
                        "at": {},
                        "pairs": {},
                        "tree": {},
                    }
                return states[c]

            def score_g(s):
                c, kb = divmod(s, NKB)
                st = get_state(c)
                st_ps = ps_s.tile([128, QC], F32, tag="st", name="st_ps")
                for h in range(QC // 512):
                    nc.tensor.matmul(
                        st_ps[:, h * 512 : (h + 1) * 512],
                        kt_sb[:, kb * 128 : (kb + 1) * 128],
                        qt_sb[:, c * QC + h * 512 : c * QC + (h + 1) * 512],
                        start=True,
                        stop=True,
                    )
                at_sb = attn_pool.tile([128, QC], F16, tag="at", name="at_sb")
                nc.scalar.activation(
                    at_sb[:],
                    st_ps[:],
                    mybir.ActivationFunctionType.Exp,
                    scale=scale,
                )
                st["at"][kb] = at_sb

            def av_g(s):
                c, kb = divmod(s, NKB)
                st = get_state(c)
                at_sb = st["at"][kb]
                for h in range(QC // 512):
                    nc.tensor.matmul(
                        st["outT"][:, h * 512 : (h + 1) * 512],
                        v_sb[:, kb, :],
                        at_sb[:, h * 512 : (h + 1) * 512],
                        start=(kb == 0),
                        stop=(kb == NKB - 1),
                    )

            def tadd(name, a, b):
                t = tree_pool.tile([128, QC], F16, tag=name, name=name)
                nc.vector.tensor_add(t[:], a[:], b[:])
                return t

            def tree_g(s):
                # Lopsided pairwise l-reduction per chunk; part3 = part2+at14
                # forms at exp(14) so only ONE DVE add trails exp(15).
                c, kb = divmod(s, NKB)
                st = get_state(c)
                at, pairs, tr = st["at"], st["pairs"], st["tree"]
                if kb % 2 == 1 and kb < 15:
                    pairs[kb // 2] = tadd("pair", at[kb - 1], at[kb])
                if kb == 3:
                    tr["quad0"] = tadd("quad0", pairs[0], pairs[1])
                if kb == 7:
                    tr["quad1"] = tadd("quad1", pairs[2], pairs[3])
                    tr["oct0"] = tadd("oct0", tr["quad0"], tr["quad1"])
                if kb == 11:
                    tr["quad2"] = tadd("quad2", pairs[4], pairs[5])
                    tr["part1"] = tadd("part1", tr["oct0"], tr["quad2"])
                if kb == 13:
                    tr["part2"] = tadd("part2", tr["part1"], pairs[6])
                if kb == 14:
                    tr["part3"] = tadd("part3", tr["part2"], at[14])
                if kb == 15:
                    final = tadd("final", tr["part3"], at[15])
                    # ship the unreduced [128, QC] tree sum; host finishes
                    nc.sync.dma_start(
                        lfin_d.ap()[:, c * QC : (c + 1) * QC], final[:]
                    )

            def evac_g(c):
                # outT -> SBUF fp16 in 512-halves so the next chunk's first
                # AV (which reuses the ps_o banks) unblocks half at a time
                st = get_state(c)
                outT_sb = fin_pool.tile([128, QC], F16, tag="outT_sb",
                                        name="outT_sb")
                for h in range(QC // 512):
                    nc.vector.tensor_copy(
                        outT_sb[:, h * 512 : (h + 1) * 512],
                        st["outT"][:, h * 512 : (h + 1) * 512],
                    )
                    nc.sync.dma_start(
                        outT_d.ap()[:, c * QC + h * 512 : c * QC + (h + 1) * 512],
                        outT_sb[:, h * 512 : (h + 1) * 512],
                    )

            # chunk-0 JIT blocks, keyed by the score(kb) they must PRECEDE:
            # k_g before score(4g); v_g before AV(4g) at slot 4g+LAG; q2/q3
            # before chunk 1.  Spread so no early slot lumps two blocks.
            work0 = {
                3: [("k", 1)],
                4: [("v", 0)],
                7: [("k", 2)],
                8: [("v", 1)],
                11: [("k", 3)],
                12: [("v", 2)],
                13: [("q", 2)],
                14: [("v", 3)],
                15: [("q", 3)],
            }
            work0 = {
                kb: [lambda item=item: do_work(item) for item in items]
                for kb, items in work0.items()
            }
            # chunks 1-2: next chunk's q blocks as 2-dc slivers AFTER the
            # score so the exp cadence is never queued behind a projection
            spread_work = {}
            for c in (1, 2):
                qa = project_spread("wq", qt_sb, 2 * c + 2)
                qb = project_spread("wq", qt_sb, 2 * c + 3)
                spread_work[c] = {4: [qa], 6: [qa], 8: [qa], 10: [qa],
                                  11: [qb], 12: [qb], 13: [qb], 14: [qb]}

            for s in range(NS + LAG):
                if s < NS:
                    c, kb = divmod(s, NKB)
                    if c == 0:
                        for thunk in work0.get(kb, ()):
                            thunk()
                    score_g(s)
                    if c in spread_work:
                        for thunk in spread_work[c].get(kb, ()):
                            thunk()
                if s >= LAG:
                    av_g(s - LAG)
                    cp, kbp = divmod(s - LAG, NKB)
                    if kbp == NKB - 1:
                        evac_g(cp)
                if 1 <= s <= NS:
                    tree_g(s - 1)

    nc.compile()
    return nc


def _get_nc():
    if "nc" not in _CACHE:
        _CACHE["nc"] = build_nc()
    return _CACHE["nc"]


def _swizzle_w(W):
    # [D, H] -> [128, NDC*H]: row p, chunk c holds W[c*128+p, :]
    W = np.asarray(W, dtype=np.float16)
    return np.ascontiguousarray(
        W.reshape(NDC, 128, H).transpose(1, 0, 2).reshape(128, NDC * H)
    )


def make_in_maps(inputs, Wq, Wk, Wv):
    inputs = np.asarray(inputs, dtype=np.float32)
    Wq = _swizzle_w(Wq)
    Wk = _swizzle_w(Wk)
    Wv = _swizzle_w(Wv)
    ident = np.eye(128, dtype=np.float16)

    in_maps = []
    for c in range(NCORES):
        b, kh = divmod(c, 2)
        xb = inputs[b]
        # own key-half rows first; queries follow the same permutation
        xk = np.concatenate(
            [xb[kh * SK : (kh + 1) * SK], xb[(1 - kh) * SK : (2 - kh) * SK]], axis=0
        )
        xt = xk.T.astype(np.float16)  # [D, S]
        # slice-contiguous staging: [g, p, c*RB+s] = xT[c*128+p, g*RB+s]
        xt8 = np.ascontiguousarray(
            xt.reshape(NDC, 128, NRB, RB)
            .transpose(2, 1, 0, 3)
            .reshape(NRB, 128, NDC * RB)
        )
        in_maps.append(
            {
                "xt": xt8,
                "wq": Wq,
                "wk": Wk,
                "wv": Wv,
                "ident": ident,
            }
        )
    return in_maps


def kernel(inputs, Wq, Wk, Wv):
    nc = _get_nc()
    in_maps = make_in_maps(inputs, Wq, Wk, Wv)

    res = run_bass_kernel_spmd(nc, in_maps, core_ids=list(range(NCORES)))

    out = np.empty((B, S, H), dtype=np.float32)
    for b in range(B):
        num = np.zeros((H, S), dtype=np.float32)
        den = np.zeros((1, S), dtype=np.float32)
        for kh in range(2):
            c = 2 * b + kh
            outT = res.results[c]["outT"].astype(np.float32)  # [H,S], permuted
            # denominators: host-side partition sum of the kernel's
            # unreduced [128, S] tree totals
            l = (
                res.results[c]["lfin"].astype(np.float32).sum(axis=0)[None, :]
            )  # [1, S]
            # queries were ordered [kh-half, other-half]; map back
            perm = np.concatenate(
                [
                    np.arange(kh * SK, (kh + 1) * SK),
                    np.arange((1 - kh) * SK, (2 - kh) * SK),
                ]
            )
            num[:, perm] += outT
            den[:, perm] += l
        out[b] = (num / den).T
    return out

